# revision 26
# baseline (speedup 1.0000x reference)
"""GCN layer (nn_GCNLayer) on 8 TRN2 NeuronCores via Bass/Tile.

Reference math (f32):
    A_hat  = A + I
    D      = A_hat.sum(axis=1)                  # = rowsum(A) + 1
    d      = 1/sqrt(D + 1e-10)
    out    = relu((d[:,None] * A_hat * d[None,:]) @ (X @ W))

Rewritten to avoid materializing A_norm:
    Ys     = d[:,None] * (X @ W)                                 # [N, C]
    out[r] = relu(d[r] * (A[r,:] @ Ys + Ys[r]))                  # +Ys[r] is the +I diag

Sharding (8 cores): rows of A ([N/8, N]) and X ([N/8, F]); W replicated.

Final design (build_v6, selected by default):
  phase 0: XW_loc = X_shard @ W (bf16 matmul, f32 accumulate)
  phase 1: stream A_shard f32 from HBM once per [128 x 1024] chunk:
           cast f32->bf16 in two halves (ScalarE + VectorE, both with fused
           free-dim accumulation => rowsums), PE-transpose the 128x128 tiles
           and keep A^T bf16 resident in SBUF (16 MB).
           As each stripe finishes, its rowsums are complete, so d and
           Ys = d*XW for those rows are computed locally and the scaled bf16
           Ys stripe is AllGathered (8 small Mesh collectives, overlapped
           with streaming; the d pipeline is emitted ahead of burst work so
           triggers stay on the critical path).
           Main-matmul "bursts" (lhsT = A^T tile, rhs = Ys tile, f32 PSUM)
           interleave with the transpose pipeline as chunks land, partials
           accumulated in SBUF.
  final:   out[r] = relu(d_r * (acc[r] + Ys_loc[r])), DMA out.

DMA queues are split by purpose (sync = A stream, scalar = outputs,
gpsimd = collective bounces) to avoid head-of-line blocking, and a
dependency-free warmup AllGather starts the collective-subsystem init at
t=0. Earlier, simpler variants (build/build_v3/build_v4) are kept for
reference; measured HW exec on the full problem: serial ~300us,
final ~257us, rel err ~3.1e-3 (bf16 compute, f32 accumulate).
"""

import os
import sys

import numpy as np

sys.path.insert(0, "/opt/trn_rl_repo")

from contextlib import ExitStack

from concourse import bacc, bass, mybir, tile
from concourse.bass_utils import run_bass_kernel_spmd
from concourse.masks import make_identity

F32 = mybir.dt.float32
BF16 = mybir.dt.bfloat16
AF = mybir.ActivationFunctionType


def _ensure_axon_ntff_hook():
    """run_bass_kernel_spmd(trace=True) under axon imports
    antenv.axon_hooks, which the container's antenv stub lacks. Provide it
    via sys.modules, driving NTFF capture through libaxon_pjrt.so ctypes."""
    try:
        import antenv.axon_hooks  # noqa: F401

        return
    except ImportError:
        pass
    import contextlib
    import ctypes
    import types

    mod = types.ModuleType("antenv.axon_hooks")
    state = {"hook": None}

    def _build(so_path):
        if not os.path.exists(so_path):
            return None
        lib = ctypes.CDLL(so_path)
        if not hasattr(lib, "axon_start_nrt_profile"):
            return None
        lib.axon_start_nrt_profile.argtypes = [
            ctypes.POINTER(ctypes.c_int64),
            ctypes.c_size_t,
        ]
        lib.axon_start_nrt_profile.restype = ctypes.c_int64
        lib.axon_stop_nrt_profile.argtypes = [ctypes.c_char_p]
        lib.axon_stop_nrt_profile.restype = ctypes.c_int64

        @contextlib.contextmanager
        def _hook(output_dir, device_ids):
            import jax

            jax.devices()
            if device_ids:
                ids = (ctypes.c_int64 * len(device_ids))(*device_ids)
                rc = lib.axon_start_nrt_profile(ids, len(device_ids))
            else:
                rc = lib.axon_start_nrt_profile(None, 0)
            if rc != 0:
                raise RuntimeError(f"axon_start_nrt_profile rc={rc}")
            try:
                yield
            finally:
                n = lib.axon_stop_nrt_profile(str(output_dir).encode())
                if n < 0:
                    raise RuntimeError(f"axon_stop_nrt_profile rc={n}")

        return _hook

    def set_axon_ntff_profile_hook(hook):
        state["hook"] = hook

    def get_axon_ntff_profile_hook():
        if state["hook"] is None:
            state["hook"] = _build(
                os.environ.get("AXON_PJRT_SO", "/opt/axon/libaxon_pjrt.so")
            )
        return state["hook"]

    mod.set_axon_ntff_profile_hook = set_axon_ntff_profile_hook
    mod.get_axon_ntff_profile_hook = get_axon_ntff_profile_hook
    sys.modules["antenv.axon_hooks"] = mod
    try:
        import antenv

        antenv.axon_hooks = mod
    except ImportError:
        pass

N, FDIM, CDIM = 8192, 512, 256
NCORES = 8


def build(n=N, fdim=FDIM, cdim=CDIM, ncores=NCORES, ch=1024):
    """Build the SPMD Bass program (identical on every core)."""
    R = n // ncores      # rows per core
    S = R // 128         # 128-row stripes per core
    KT = n // 128        # contraction tiles
    NCH = n // ch        # chunks per stripe
    FT = fdim // 128
    KPC = KT // ncores   # k-tiles owned per core (= S)
    assert KPC == S

    nc = bacc.Bacc(
        "TRN2", target_bir_lowering=False, debug=False, num_devices=ncores
    )
    A_d = nc.dram_tensor("A", [R, n], F32, kind="ExternalInput").ap()
    X_d = nc.dram_tensor("X", [R, fdim], F32, kind="ExternalInput").ap()
    W_d = nc.dram_tensor("W", [fdim, cdim], F32, kind="ExternalInput").ap()
    out_d = nc.dram_tensor("out", [R, cdim], F32, kind="ExternalOutput").ap()
    xw_in_d = nc.dram_tensor("xw_in", [R, cdim], BF16).ap()
    xw_out_d = nc.dram_tensor("xw_out", [n, cdim], BF16, addr_space="Shared").ap()
    d_in_d = nc.dram_tensor("d_in", [R], F32).ap()
    d_out_d = nc.dram_tensor("d_out", [n], F32, addr_space="Shared").ap()
    groups = [list(range(ncores))]

    with tile.TileContext(nc) as tc, ExitStack() as ctx:
        const_pool = ctx.enter_context(tc.tile_pool(name="const", bufs=1))
        ident = const_pool.tile([128, 128], BF16)
        make_identity(nc, ident[:])
        ident_f = const_pool.tile([128, 128], F32)
        make_identity(nc, ident_f[:])

        # Persistent big tensors.
        at_pool = ctx.enter_context(tc.tile_pool(name="atp", bufs=1))
        # A^T bf16, stripe-major: slice (s, kt) at free offset (s*KT + kt)*128
        AT = at_pool.tile([128, S * KT * 128], BF16)
        ys_pool = ctx.enter_context(tc.tile_pool(name="ysp", bufs=1))
        ys_sb = ys_pool.tile([128, KT * cdim], BF16)   # Ys, kt-major

        small_pool = ctx.enter_context(tc.tile_pool(name="small", bufs=1))
        xw_f32 = small_pool.tile([128, S * cdim], F32)   # XW_loc then Ys_loc (in place)
        xw_bf = small_pool.tile([128, S * cdim], BF16)
        Dacc = small_pool.tile([128, S * NCH], F32)
        Dsum = small_pool.tile([128, S], F32)
        d_loc = small_pool.tile([128, S], F32)
        d_kt = small_pool.tile([128, KT], F32)
        dT_sb = small_pool.tile([128, 128], F32)
        dg_sb = small_pool.tile([128, 128], F32)

        # ---- Phase 0: XW_loc = X_shard @ W (bf16) ----
        with tc.tile_pool(name="ph0", bufs=2) as ph0, \
             tc.tile_pool(name="ph0c", bufs=1) as ph0c, \
             tc.tile_pool(name="ph0ps", bufs=2, space="PSUM") as ph0ps:
            w_f32 = ph0c.tile([128, FT * cdim], F32)
            w_bf = ph0c.tile([128, FT * cdim], BF16)
            for f in range(FT):
                nc.sync.dma_start(
                    w_f32[:, f * cdim:(f + 1) * cdim],
                    W_d[f * 128:(f + 1) * 128, :],
                )
            nc.vector.tensor_copy(w_bf[:], w_f32[:])

            xT = ph0c.tile([128, S * FT * 128], BF16)  # X^T tiles, (s, f)
            for s in range(S):
                x_f32 = ph0.tile([128, fdim], F32)
                nc.sync.dma_start(x_f32[:], X_d[s * 128:(s + 1) * 128, :])
                x_bf = ph0.tile([128, fdim], BF16)
                nc.vector.tensor_copy(x_bf[:], x_f32[:])
                pxt = ph0ps.tile([128, fdim], BF16)
                for f in range(FT):
                    nc.tensor.transpose(
                        pxt[:, f * 128:(f + 1) * 128],
                        x_bf[:, f * 128:(f + 1) * 128],
                        ident[:],
                    )
                nc.scalar.copy(
                    xT[:, (s * FT) * 128:(s * FT + FT) * 128], pxt[:]
                )
            for s in range(S):
                pxw = ph0ps.tile([128, cdim], F32)
                for f in range(FT):
                    nc.tensor.matmul(
                        pxw[:],
                        lhsT=xT[:, (s * FT + f) * 128:(s * FT + f + 1) * 128],
                        rhs=w_bf[:, f * cdim:(f + 1) * cdim],
                        start=(f == 0),
                        stop=(f == FT - 1),
                    )
                nc.vector.tensor_copy(xw_f32[:, s * cdim:(s + 1) * cdim], pxw[:])

        # Early AllGather of (unscaled) XW in bf16, overlapped with phase 1.
        nc.vector.tensor_copy(xw_bf[:], xw_f32[:])
        nc.gpsimd.dma_start(
            xw_in_d.rearrange("(s p) c -> p s c", p=128),
            xw_bf[:].rearrange("p (s c) -> p s c", s=S),
        )
        nc.gpsimd.collective_compute(
            "AllGather",
            mybir.AluOpType.bypass,
            replica_groups=groups,
            ins=[xw_in_d],
            outs=[xw_out_d],
        )
        KCH = KT // 8
        for b in range(8):
            nc.gpsimd.dma_start(
                ys_sb[:, b * KCH * cdim:(b + 1) * KCH * cdim].rearrange(
                    "p (k c) -> p k c", k=KCH
                ),
                xw_out_d[b * KCH * 128:(b + 1) * KCH * 128, :].rearrange(
                    "(k p) c -> p k c", p=128
                ),
            )

        # ---- Phase 1: stream A, cast+rowsum, transpose into resident A^T ----
        TPC = ch // 128          # transposes per chunk
        GRP = 8                  # transposes per PSUM bank / drain
        with tc.tile_pool(name="ast", bufs=3) as ast, \
             tc.tile_pool(name="abf", bufs=4) as abf, \
             tc.tile_pool(name="tps", bufs=3, space="PSUM") as tps:
            for s in range(S):
                for c in range(NCH):
                    a_ch = ast.tile([128, ch], F32)
                    nc.sync.dma_start(
                        a_ch[:], A_d[s * 128:(s + 1) * 128, c * ch:(c + 1) * ch]
                    )
                    a_bf = abf.tile([128, ch], BF16)
                    i = s * NCH + c
                    # cast f32->bf16 with fused free-dim accumulation (rowsum),
                    # alternating engines so neither ACT nor DVE bottlenecks
                    if i % 2 == 0:
                        nc.scalar.activation(
                            a_bf[:], a_ch[:], AF.Copy, accum_out=Dacc[:, i:i + 1]
                        )
                    else:
                        nc.vector.tensor_scalar(
                            a_bf[:], a_ch[:], 1.0, 0.0,
                            mybir.AluOpType.mult,
                            mybir.AluOpType.add,
                            accum_out=Dacc[:, i:i + 1],
                        )
                    for g in range(TPC // GRP):
                        pt = tps.tile([128, GRP * 128], BF16)
                        for t in range(GRP):
                            nc.tensor.transpose(
                                pt[:, t * 128:(t + 1) * 128],
                                a_bf[:, (g * GRP + t) * 128:(g * GRP + t + 1) * 128],
                                ident[:],
                            )
                        kt0 = c * TPC + g * GRP
                        dst = AT[:, (s * KT + kt0) * 128:(s * KT + kt0 + GRP) * 128]
                        nc.vector.tensor_copy(dst, pt[:])

        # ---- Boundary: d locally, tiny d AllGather, scale Ys ----
        nc.vector.tensor_reduce(
            Dsum[:],
            Dacc[:].rearrange("p (s c) -> p s c", s=S),
            axis=mybir.AxisListType.X,
            op=mybir.AluOpType.add,
        )
        # Dsq = sqrt(D + 1 + 1e-10); d = 1/Dsq
        Dsq = small_pool.tile([128, S], F32)
        bias1 = small_pool.tile([128, 1], F32)
        nc.gpsimd.memset(bias1[:], 1.0 + 1e-10)
        nc.scalar.activation(Dsq[:], Dsum[:], AF.Sqrt, bias=bias1[:])
        nc.vector.reciprocal(d_loc[:], Dsq[:])
        # Ys_loc = d * XW_loc (in place, f32) for the +I diagonal term
        for s in range(S):
            nc.vector.tensor_scalar_mul(
                xw_f32[:, s * cdim:(s + 1) * cdim],
                xw_f32[:, s * cdim:(s + 1) * cdim],
                d_loc[:, s:s + 1],
            )
        # d_loc [128, S] -> transposed [S, 128] -> DRAM [R] in global row order
        with tc.tile_pool(name="bps", bufs=2, space="PSUM") as bps:
            pdT = bps.tile([S, 128], F32)
            nc.tensor.transpose(pdT[:], d_loc[:], ident_f[:])
            nc.vector.tensor_copy(dT_sb[:S, :], pdT[:])
            nc.scalar.dma_start(d_in_d.rearrange("(s p) -> s p", p=128), dT_sb[:S, :])
            nc.gpsimd.collective_compute(
                "AllGather",
                mybir.AluOpType.bypass,
                replica_groups=groups,
                ins=[d_in_d],
                outs=[d_out_d],
            )
            # d_out [n] -> [KT, 128] sbuf -> transpose -> d_kt [128, KT]
            nc.scalar.dma_start(
                dg_sb[:KT, :], d_out_d.rearrange("(m p) -> m p", p=128)
            )
            pdg = bps.tile([128, KT], F32)
            nc.tensor.transpose(pdg[:], dg_sb[:KT, :], ident_f[:KT, :KT])
            nc.vector.tensor_copy(d_kt[:], pdg[:])
        # Scale the gathered XW by d[k] per k-tile (in place, bf16)
        for kt in range(KT):
            nc.vector.tensor_scalar_mul(
                ys_sb[:, kt * cdim:(kt + 1) * cdim],
                ys_sb[:, kt * cdim:(kt + 1) * cdim],
                d_kt[:, kt:kt + 1],
            )

        # ---- Phase 2: out[s] = relu(d * (A_shard @ Ys + Ys_loc[s])) ----
        with tc.tile_pool(name="ops", bufs=2, space="PSUM") as ops, \
             tc.tile_pool(name="outp", bufs=3) as outp:
            for s in range(S):
                po = ops.tile([128, cdim], F32)
                for kt in range(KT):
                    nc.tensor.matmul(
                        po[:],
                        lhsT=AT[:, (s * KT + kt) * 128:(s * KT + kt + 1) * 128],
                        rhs=ys_sb[:, kt * cdim:(kt + 1) * cdim],
                        start=(kt == 0),
                        stop=(kt == KT - 1),
                    )
                nc.vector.tensor_add(
                    po[:], po[:], xw_f32[:, s * cdim:(s + 1) * cdim]
                )
                ot = outp.tile([128, cdim], F32)
                nc.scalar.activation(ot[:], po[:], AF.Relu, scale=d_loc[:, s:s + 1])
                nc.scalar.dma_start(out_d[s * 128:(s + 1) * 128, :], ot[:])

    nc.compile()
    return nc


def build_v3(n=N, fdim=FDIM, cdim=CDIM, ncores=NCORES, ch=1024):
    """Overlapped variant: chunked d AllGathers let the main matmuls run
    interleaved with the A streaming/transpose phase instead of after it.

    Phase-2 work is emitted in 'bursts' as (A^T stripes, d chunks) become
    available, accumulating partial sums in SBUF (PSUM zero-regions are
    bank-granular, so 8 concurrent open groups don't fit alongside the
    transpose banks)."""
    R = n // ncores
    S = R // 128
    KT = n // 128
    NCH = n // ch
    FT = fdim // 128
    assert KT // ncores == S
    DCH = min(4, S)          # d-exchange chunks
    SPC = S // DCH           # stripes per d-chunk
    assert S % DCH == 0

    nc = bacc.Bacc(
        "TRN2", target_bir_lowering=False, debug=False, num_devices=ncores
    )
    A_d = nc.dram_tensor("A", [R, n], F32, kind="ExternalInput").ap()
    X_d = nc.dram_tensor("X", [R, fdim], F32, kind="ExternalInput").ap()
    W_d = nc.dram_tensor("W", [fdim, cdim], F32, kind="ExternalInput").ap()
    out_d = nc.dram_tensor("out", [R, cdim], F32, kind="ExternalOutput").ap()
    xw_in_d = nc.dram_tensor("xw_in", [R, cdim], BF16).ap()
    xw_out_d = nc.dram_tensor("xw_out", [n, cdim], BF16, addr_space="Shared").ap()
    d_in_d = [
        nc.dram_tensor(f"d_in{c}", [SPC * 128], F32).ap() for c in range(DCH)
    ]
    d_out_d = [
        nc.dram_tensor(
            f"d_out{c}", [ncores * SPC * 128], F32, addr_space="Shared"
        ).ap()
        for c in range(DCH)
    ]
    groups = [list(range(ncores))]

    with tile.TileContext(nc) as tc, ExitStack() as ctx:
        const_pool = ctx.enter_context(tc.tile_pool(name="const", bufs=1))
        ident = const_pool.tile([128, 128], BF16)
        make_identity(nc, ident[:])
        ident_f = const_pool.tile([128, 128], F32)
        make_identity(nc, ident_f[:])

        at_pool = ctx.enter_context(tc.tile_pool(name="atp", bufs=1))
        AT = at_pool.tile([128, S * KT * 128], BF16)
        ys_pool = ctx.enter_context(tc.tile_pool(name="ysp", bufs=1))
        ys_sb = ys_pool.tile([128, KT * cdim], BF16)

        small_pool = ctx.enter_context(tc.tile_pool(name="small", bufs=1))
        xw_f32 = small_pool.tile([128, S * cdim], F32)
        xw_bf = small_pool.tile([128, S * cdim], BF16)
        acc_sb = small_pool.tile([128, S * cdim], F32)
        Dacc = small_pool.tile([128, S * NCH], F32)
        Dsum = small_pool.tile([128, S], F32)
        Dsq = small_pool.tile([128, S], F32)
        d_loc = small_pool.tile([128, S], F32)
        d_kt = small_pool.tile([128, DCH * ncores * SPC], F32)
        dT_sb = small_pool.tile([128, 128], F32)
        dg_sb = small_pool.tile([128, 128], F32)
        bias1 = small_pool.tile([128, 1], F32)
        nc.gpsimd.memset(bias1[:], 1.0 + 1e-10)

        # ---- Phase 0: XW_loc = X_shard @ W (bf16) ----
        with tc.tile_pool(name="ph0", bufs=2) as ph0, \
             tc.tile_pool(name="ph0c", bufs=1) as ph0c, \
             tc.tile_pool(name="ph0ps", bufs=2, space="PSUM") as ph0ps:
            w_f32 = ph0c.tile([128, FT * cdim], F32)
            w_bf = ph0c.tile([128, FT * cdim], BF16)
            for f in range(FT):
                nc.sync.dma_start(
                    w_f32[:, f * cdim:(f + 1) * cdim],
                    W_d[f * 128:(f + 1) * 128, :],
                )
            nc.vector.tensor_copy(w_bf[:], w_f32[:])
            xT = ph0c.tile([128, S * FT * 128], BF16)
            for s in range(S):
                x_f32 = ph0.tile([128, fdim], F32)
                nc.sync.dma_start(x_f32[:], X_d[s * 128:(s + 1) * 128, :])
                x_bf = ph0.tile([128, fdim], BF16)
                nc.vector.tensor_copy(x_bf[:], x_f32[:])
                pxt = ph0ps.tile([128, fdim], BF16)
                for f in range(FT):
                    nc.tensor.transpose(
                        pxt[:, f * 128:(f + 1) * 128],
                        x_bf[:, f * 128:(f + 1) * 128],
                        ident[:],
                    )
                nc.scalar.copy(xT[:, (s * FT) * 128:(s * FT + FT) * 128], pxt[:])
            for s in range(S):
                pxw = ph0ps.tile([128, cdim], F32)
                for f in range(FT):
                    nc.tensor.matmul(
                        pxw[:],
                        lhsT=xT[:, (s * FT + f) * 128:(s * FT + f + 1) * 128],
                        rhs=w_bf[:, f * cdim:(f + 1) * cdim],
                        start=(f == 0),
                        stop=(f == FT - 1),
                    )
                nc.vector.tensor_copy(xw_f32[:, s * cdim:(s + 1) * cdim], pxw[:])

        # Early AllGather of (unscaled) XW in bf16.
        nc.vector.tensor_copy(xw_bf[:], xw_f32[:])
        nc.gpsimd.dma_start(
            xw_in_d.rearrange("(s p) c -> p s c", p=128),
            xw_bf[:].rearrange("p (s c) -> p s c", s=S),
        )
        nc.gpsimd.collective_compute(
            "AllGather",
            mybir.AluOpType.bypass,
            replica_groups=groups,
            ins=[xw_in_d],
            outs=[xw_out_d],
        )
        KCH = KT // 8
        for b in range(8):
            nc.gpsimd.dma_start(
                ys_sb[:, b * KCH * cdim:(b + 1) * KCH * cdim].rearrange(
                    "p (k c) -> p k c", k=KCH
                ),
                xw_out_d[b * KCH * 128:(b + 1) * KCH * 128, :].rearrange(
                    "(k p) c -> p k c", p=128
                ),
            )

        # ---- Phase 1 + interleaved phase 2 bursts ----
        TPC = ch // 128
        GRP = 8
        ready_kts: list = []
        burst_n = [0] * S

        with tc.tile_pool(name="ast", bufs=3) as ast, \
             tc.tile_pool(name="abf", bufs=3) as abf, \
             tc.tile_pool(name="bps", bufs=1, space="PSUM") as bps, \
             tc.tile_pool(name="bur", bufs=2, space="PSUM") as bur, \
             tc.tile_pool(name="tps", bufs=2, space="PSUM") as tps, \
             tc.tile_pool(name="outp", bufs=3) as outp:
            def emit_burst(r, kts):
                if not kts:
                    return
                pb = bur.tile([128, cdim], F32, tag="burst")
                for idx, kt in enumerate(kts):
                    nc.tensor.matmul(
                        pb[:],
                        lhsT=AT[:, (r * KT + kt) * 128:(r * KT + kt + 1) * 128],
                        rhs=ys_sb[:, kt * cdim:(kt + 1) * cdim],
                        start=(idx == 0),
                        stop=(idx == len(kts) - 1),
                    )
                dst = acc_sb[:, r * cdim:(r + 1) * cdim]
                if burst_n[r] == 0:
                    nc.vector.tensor_copy(dst, pb[:])
                else:
                    nc.vector.tensor_add(dst, dst, pb[:])
                burst_n[r] += 1

            for s in range(S):
                for c in range(NCH):
                    a_ch = ast.tile([128, ch], F32)
                    nc.sync.dma_start(
                        a_ch[:], A_d[s * 128:(s + 1) * 128, c * ch:(c + 1) * ch]
                    )
                    a_bf = abf.tile([128, ch], BF16)
                    i = s * NCH + c
                    if i % 2 == 0:
                        nc.scalar.activation(
                            a_bf[:], a_ch[:], AF.Copy, accum_out=Dacc[:, i:i + 1]
                        )
                    else:
                        nc.vector.tensor_scalar(
                            a_bf[:], a_ch[:], 1.0, 0.0,
                            mybir.AluOpType.mult,
                            mybir.AluOpType.add,
                            accum_out=Dacc[:, i:i + 1],
                        )
                    for g in range(TPC // GRP):
                        pt = tps.tile([128, GRP * 128], BF16)
                        for t in range(GRP):
                            nc.tensor.transpose(
                                pt[:, t * 128:(t + 1) * 128],
                                a_bf[:, (g * GRP + t) * 128:(g * GRP + t + 1) * 128],
                                ident[:],
                            )
                        kt0 = c * TPC + g * GRP
                        dst = AT[:, (s * KT + kt0) * 128:(s * KT + kt0 + GRP) * 128]
                        nc.vector.tensor_copy(dst, pt[:])

                # New stripe ready: burst it against previously-ready k-tiles.
                emit_burst(s, ready_kts)

                if (s + 1) % SPC == 0:
                    dc = s // SPC
                    s0 = dc * SPC
                    # Local d for stripes [s0, s0+SPC)
                    nc.vector.tensor_reduce(
                        Dsum[:, s0:s0 + SPC],
                        Dacc[:, s0 * NCH:(s0 + SPC) * NCH].rearrange(
                            "p (s c) -> p s c", s=SPC
                        ),
                        axis=mybir.AxisListType.X,
                        op=mybir.AluOpType.add,
                    )
                    nc.scalar.activation(
                        Dsq[:, s0:s0 + SPC], Dsum[:, s0:s0 + SPC],
                        AF.Sqrt, bias=bias1[:],
                    )
                    nc.vector.reciprocal(
                        d_loc[:, s0:s0 + SPC], Dsq[:, s0:s0 + SPC]
                    )
                    # Ys_loc slice (diagonal term), in place f32
                    for sl in range(s0, s0 + SPC):
                        nc.vector.tensor_scalar_mul(
                            xw_f32[:, sl * cdim:(sl + 1) * cdim],
                            xw_f32[:, sl * cdim:(sl + 1) * cdim],
                            d_loc[:, sl:sl + 1],
                        )
                    # Exchange this chunk of d
                    pdT = bps.tile([SPC, 128], F32, tag="pdT")
                    nc.tensor.transpose(pdT[:], d_loc[:, s0:s0 + SPC], ident_f[:])
                    nc.vector.tensor_copy(dT_sb[:SPC, :], pdT[:])
                    nc.scalar.dma_start(
                        d_in_d[dc].rearrange("(s p) -> s p", p=128),
                        dT_sb[:SPC, :],
                    )
                    nc.gpsimd.collective_compute(
                        "AllGather",
                        mybir.AluOpType.bypass,
                        replica_groups=groups,
                        ins=[d_in_d[dc]],
                        outs=[d_out_d[dc]],
                    )
                    M = ncores * SPC
                    nc.scalar.dma_start(
                        dg_sb[:M, :], d_out_d[dc].rearrange("(m p) -> m p", p=128)
                    )
                    pdg = bps.tile([128, M], F32, tag="pdg")
                    nc.tensor.transpose(pdg[:], dg_sb[:M, :], ident_f[:M, :M])
                    nc.vector.tensor_copy(d_kt[:, dc * M:(dc + 1) * M], pdg[:])
                    # Scale the gathered XW tiles this chunk covers
                    new_kts = []
                    for m in range(M):
                        kt = (m // SPC) * S + s0 + (m % SPC)
                        new_kts.append(kt)
                        nc.vector.tensor_scalar_mul(
                            ys_sb[:, kt * cdim:(kt + 1) * cdim],
                            ys_sb[:, kt * cdim:(kt + 1) * cdim],
                            d_kt[:, dc * M + m:dc * M + m + 1],
                        )
                    # Burst the new k-tiles against all streamed stripes
                    for r in range(s + 1):
                        emit_burst(r, new_kts)
                    ready_kts.extend(new_kts)

            # ---- Final: out[r] = relu(d * (acc + Ys_loc[r])) ----
            for r in range(S):
                assert burst_n[r] > 0
                dst = acc_sb[:, r * cdim:(r + 1) * cdim]
                nc.vector.tensor_add(dst, dst, xw_f32[:, r * cdim:(r + 1) * cdim])
                ot = outp.tile([128, cdim], F32)
                nc.scalar.activation(ot[:], dst, AF.Relu, scale=d_loc[:, r:r + 1])
                nc.scalar.dma_start(out_d[r * 128:(r + 1) * 128, :], ot[:])

    nc.compile()
    return nc



def build_v4(n=N, fdim=FDIM, cdim=CDIM, ncores=NCORES, ch=1024):
    """V4: overlapped design with stall fixes.

    - warmup collective absorbs the ~30us first-collective setup lag
    - DMA queue split: A-stream on sync HWDGE; ys/xw/out on scalar HWDGE;
      d gather-back + collectives on gpsimd (SW DGE); tiny d outbound on sync
    - per-chunk cast split in halves (ACT one, DVE other) to shorten the
      DMA->cast->transpose latency chain
    - burst/post-AllGather work emitted one stripe late so the in-order
      engine streams don't head-of-line block on collective latency
    """
    R = n // ncores
    S = R // 128
    KT = n // 128
    NCH = n // ch
    FT = fdim // 128
    assert KT // ncores == S
    DCH = min(4, S)
    SPC = S // DCH
    assert S % DCH == 0
    HACC = 2 * NCH           # accum slots per stripe (2 per chunk)

    nc = bacc.Bacc(
        "TRN2", target_bir_lowering=False, debug=False, num_devices=ncores
    )
    A_d = nc.dram_tensor("A", [R, n], F32, kind="ExternalInput").ap()
    X_d = nc.dram_tensor("X", [R, fdim], F32, kind="ExternalInput").ap()
    W_d = nc.dram_tensor("W", [fdim, cdim], F32, kind="ExternalInput").ap()
    out_d = nc.dram_tensor("out", [R, cdim], F32, kind="ExternalOutput").ap()
    xw_in_d = nc.dram_tensor("xw_in", [R, cdim], BF16).ap()
    xw_out_d = nc.dram_tensor("xw_out", [n, cdim], BF16, addr_space="Shared").ap()
    wu_in_d = nc.dram_tensor("wu_in", [128], F32).ap()
    wu_out_d = nc.dram_tensor("wu_out", [ncores * 128], F32, addr_space="Shared").ap()
    d_in_d = [
        nc.dram_tensor(f"d_in{c}", [SPC * 128], F32).ap() for c in range(DCH)
    ]
    d_out_d = [
        nc.dram_tensor(
            f"d_out{c}", [ncores * SPC * 128], F32, addr_space="Shared"
        ).ap()
        for c in range(DCH)
    ]
    groups = [list(range(ncores))]

    with tile.TileContext(nc) as tc, ExitStack() as ctx:
        const_pool = ctx.enter_context(tc.tile_pool(name="const", bufs=1))
        ident = const_pool.tile([128, 128], BF16)
        make_identity(nc, ident[:])
        ident_f = const_pool.tile([128, 128], F32)
        make_identity(nc, ident_f[:])

        at_pool = ctx.enter_context(tc.tile_pool(name="atp", bufs=1))
        AT = at_pool.tile([128, S * KT * 128], BF16)
        ys_pool = ctx.enter_context(tc.tile_pool(name="ysp", bufs=1))
        ys_sb = ys_pool.tile([128, KT * cdim], BF16)

        small_pool = ctx.enter_context(tc.tile_pool(name="small", bufs=1))
        xw_f32 = small_pool.tile([128, S * cdim], F32)
        xw_bf = small_pool.tile([128, S * cdim], BF16)
        acc_sb = small_pool.tile([128, S * cdim], F32)
        Dacc = small_pool.tile([128, S * HACC], F32)
        Dsum = small_pool.tile([128, S], F32)
        Dsq = small_pool.tile([128, S], F32)
        d_loc = small_pool.tile([128, S], F32)
        d_kt = small_pool.tile([128, DCH * ncores * SPC], F32)
        dT_sb = small_pool.tile([128, 128], F32)
        dg_sb = small_pool.tile([128, 128], F32)
        bias1 = small_pool.tile([128, 1], F32)
        nc.gpsimd.memset(bias1[:], 1.0 + 1e-10)

        # Warmup collective: absorbs ncfw/NCCL first-collective setup cost.
        # Reads an unwritten internal DRAM tensor on purpose: zero input
        # dependencies, so it triggers immediately at kernel start.
        nc.gpsimd.collective_compute(
            "AllGather",
            mybir.AluOpType.bypass,
            replica_groups=groups,
            ins=[wu_in_d],
            outs=[wu_out_d],
        )

        # ---- Phase 0: XW_loc = X_shard @ W (bf16) ----
        with tc.tile_pool(name="ph0", bufs=2) as ph0, \
             tc.tile_pool(name="ph0c", bufs=1) as ph0c, \
             tc.tile_pool(name="ph0ps", bufs=2, space="PSUM") as ph0ps:
            w_f32 = ph0c.tile([128, FT * cdim], F32)
            w_bf = ph0c.tile([128, FT * cdim], BF16)
            for f in range(FT):
                nc.sync.dma_start(
                    w_f32[:, f * cdim:(f + 1) * cdim],
                    W_d[f * 128:(f + 1) * 128, :],
                )
            nc.vector.tensor_copy(w_bf[:], w_f32[:])
            xT = ph0c.tile([128, S * FT * 128], BF16)
            for s in range(S):
                x_f32 = ph0.tile([128, fdim], F32)
                nc.sync.dma_start(x_f32[:], X_d[s * 128:(s + 1) * 128, :])
                x_bf = ph0.tile([128, fdim], BF16)
                nc.vector.tensor_copy(x_bf[:], x_f32[:])
                pxt = ph0ps.tile([128, fdim], BF16)
                for f in range(FT):
                    nc.tensor.transpose(
                        pxt[:, f * 128:(f + 1) * 128],
                        x_bf[:, f * 128:(f + 1) * 128],
                        ident[:],
                    )
                nc.scalar.copy(xT[:, (s * FT) * 128:(s * FT + FT) * 128], pxt[:])
            for s in range(S):
                pxw = ph0ps.tile([128, cdim], F32)
                for f in range(FT):
                    nc.tensor.matmul(
                        pxw[:],
                        lhsT=xT[:, (s * FT + f) * 128:(s * FT + f + 1) * 128],
                        rhs=w_bf[:, f * cdim:(f + 1) * cdim],
                        start=(f == 0),
                        stop=(f == FT - 1),
                    )
                nc.vector.tensor_copy(xw_f32[:, s * cdim:(s + 1) * cdim], pxw[:])

        # Early AllGather of (unscaled) XW in bf16; bounce DMAs on the
        # scalar HWDGE queue so the A-stream (sync queue) never waits.
        nc.vector.tensor_copy(xw_bf[:], xw_f32[:])
        nc.gpsimd.dma_start(
            xw_in_d.rearrange("(s p) c -> p s c", p=128),
            xw_bf[:].rearrange("p (s c) -> p s c", s=S),
        )
        nc.gpsimd.collective_compute(
            "AllGather",
            mybir.AluOpType.bypass,
            replica_groups=groups,
            ins=[xw_in_d],
            outs=[xw_out_d],
        )
        KCH = KT // 8
        for b in range(8):
            nc.gpsimd.dma_start(
                ys_sb[:, b * KCH * cdim:(b + 1) * KCH * cdim].rearrange(
                    "p (k c) -> p k c", k=KCH
                ),
                xw_out_d[b * KCH * 128:(b + 1) * KCH * 128, :].rearrange(
                    "(k p) c -> p k c", p=128
                ),
            )

        # ---- Phase 1 with interleaved phase-2 bursts ----
        TPC = ch // 128
        ready_kts: list = []
        burst_n = [0] * S
        pending = []             # deferred post-AllGather work (1-stripe delay)

        with tc.tile_pool(name="ast", bufs=4) as ast, \
             tc.tile_pool(name="abf", bufs=3) as abf, \
             tc.tile_pool(name="bps", bufs=1, space="PSUM") as bps, \
             tc.tile_pool(name="bur", bufs=2, space="PSUM") as bur, \
             tc.tile_pool(name="tps", bufs=3, space="PSUM") as tps, \
             tc.tile_pool(name="outp", bufs=2) as outp:

            def emit_burst(r, kts):
                if not kts:
                    return
                pb = bur.tile([128, cdim], F32, tag="burst")
                for idx, kt in enumerate(kts):
                    nc.tensor.matmul(
                        pb[:],
                        lhsT=AT[:, (r * KT + kt) * 128:(r * KT + kt + 1) * 128],
                        rhs=ys_sb[:, kt * cdim:(kt + 1) * cdim],
                        start=(idx == 0),
                        stop=(idx == len(kts) - 1),
                    )
                dst = acc_sb[:, r * cdim:(r + 1) * cdim]
                if burst_n[r] == 0:
                    nc.vector.tensor_copy(dst, pb[:])
                else:
                    nc.vector.tensor_add(dst, dst, pb[:])
                burst_n[r] += 1

            def emit_dchunk_post(dc, s_hi):
                """Post-AllGather work for d-chunk dc: gather-back, scale the
                Ys tiles it covers, burst them against stripes 0..s_hi."""
                M = ncores * SPC
                nc.gpsimd.dma_start(
                    dg_sb[:M, :], d_out_d[dc].rearrange("(m p) -> m p", p=128)
                )
                pdg = bps.tile([128, ncores * SPC], F32, tag="pdg")
                nc.tensor.transpose(pdg[:], dg_sb[:M, :], ident_f[:M, :M])
                nc.vector.tensor_copy(d_kt[:, dc * M:(dc + 1) * M], pdg[:])
                new_kts = []
                for m in range(M):
                    kt = (m // SPC) * S + dc * SPC + (m % SPC)
                    new_kts.append(kt)
                    nc.vector.tensor_scalar_mul(
                        ys_sb[:, kt * cdim:(kt + 1) * cdim],
                        ys_sb[:, kt * cdim:(kt + 1) * cdim],
                        d_kt[:, dc * M + m:dc * M + m + 1],
                    )
                for r in range(s_hi + 1):
                    emit_burst(r, new_kts)
                ready_kts.extend(new_kts)

            for s in range(S):
                for c in range(NCH):
                    a_ch = ast.tile([128, ch], F32)
                    nc.sync.dma_start(
                        a_ch[:], A_d[s * 128:(s + 1) * 128, c * ch:(c + 1) * ch]
                    )
                    a_bf = abf.tile([128, ch], BF16)
                    i = s * NCH + c
                    # cast f32->bf16 with fused rowsum accumulation, in two
                    # halves (ACT + DVE) to shorten the latency chain
                    H = ch // 2
                    nc.scalar.activation(
                        a_bf[:, :H], a_ch[:, :H], AF.Copy,
                        accum_out=Dacc[:, 2 * i:2 * i + 1],
                    )
                    nc.vector.tensor_scalar(
                        a_bf[:, H:], a_ch[:, H:], 1.0, 0.0,
                        mybir.AluOpType.mult,
                        mybir.AluOpType.add,
                        accum_out=Dacc[:, 2 * i + 1:2 * i + 2],
                    )
                    pt = tps.tile([128, TPC * 128], BF16)
                    for t in range(TPC):
                        nc.tensor.transpose(
                            pt[:, t * 128:(t + 1) * 128],
                            a_bf[:, t * 128:(t + 1) * 128],
                            ident[:],
                        )
                    kt0 = c * TPC
                    dst = AT[:, (s * KT + kt0) * 128:(s * KT + kt0 + TPC) * 128]
                    if i % 2 == 0:
                        nc.vector.tensor_copy(dst, pt[:])
                    else:
                        nc.scalar.copy(dst, pt[:])

                # New stripe against previously-ready k-tiles FIRST (the
                # deferred post below extends ready_kts with new ones).
                emit_burst(s, ready_kts)

                # Deferred post-AG work from the previous d-chunk (slack so
                # the engines reach it after the collective completed).
                if pending:
                    dc = pending.pop()
                    emit_dchunk_post(dc, s)

                if (s + 1) % SPC == 0:
                    dc = s // SPC
                    s0 = dc * SPC
                    nc.vector.tensor_reduce(
                        Dsum[:, s0:s0 + SPC],
                        Dacc[:, s0 * HACC:(s0 + SPC) * HACC].rearrange(
                            "p (s c) -> p s c", s=SPC
                        ),
                        axis=mybir.AxisListType.X,
                        op=mybir.AluOpType.add,
                    )
                    nc.scalar.activation(
                        Dsq[:, s0:s0 + SPC], Dsum[:, s0:s0 + SPC],
                        AF.Sqrt, bias=bias1[:],
                    )
                    nc.vector.reciprocal(
                        d_loc[:, s0:s0 + SPC], Dsq[:, s0:s0 + SPC]
                    )
                    for sl in range(s0, s0 + SPC):
                        nc.vector.tensor_scalar_mul(
                            xw_f32[:, sl * cdim:(sl + 1) * cdim],
                            xw_f32[:, sl * cdim:(sl + 1) * cdim],
                            d_loc[:, sl:sl + 1],
                        )
                    pdT = bps.tile([SPC, 128], F32, tag="pdT")
                    nc.tensor.transpose(pdT[:], d_loc[:, s0:s0 + SPC], ident_f[:])
                    nc.vector.tensor_copy(dT_sb[:SPC, :], pdT[:])
                    nc.sync.dma_start(
                        d_in_d[dc].rearrange("(s p) -> s p", p=128),
                        dT_sb[:SPC, :],
                    )
                    nc.gpsimd.collective_compute(
                        "AllGather",
                        mybir.AluOpType.bypass,
                        replica_groups=groups,
                        ins=[d_in_d[dc]],
                        outs=[d_out_d[dc]],
                    )
                    if s == S - 1:
                        emit_dchunk_post(dc, s)
                    else:
                        pending.append(dc)

            # ---- Final: out[r] = relu(d * (acc + Ys_loc[r])) ----
            for r in range(S):
                assert burst_n[r] > 0, f"no bursts for stripe {r}"
                dst = acc_sb[:, r * cdim:(r + 1) * cdim]
                nc.vector.tensor_add(dst, dst, xw_f32[:, r * cdim:(r + 1) * cdim])
                ot = outp.tile([128, cdim], F32)
                nc.scalar.activation(ot[:], dst, AF.Relu, scale=d_loc[:, r:r + 1])
                nc.scalar.dma_start(out_d[r * 128:(r + 1) * 128, :], ot[:])

    nc.compile()
    return nc



def build_v6(n=N, fdim=FDIM, cdim=CDIM, ncores=NCORES, ch=1024, delay=6):
    """V6: chunked Ys AllGathers.

    As each pair of stripes finishes streaming, its rowsums are complete, so
    d (and Ys = d*XW) for those rows is computable LOCALLY. AllGather the
    scaled bf16 Ys chunk directly -- no XW gather, no d exchange, no
    post-collective transposes or per-tile rescales. Main-matmul 'bursts'
    interleave with the streaming/transpose pipeline as chunks land.
    """
    R = n // ncores
    S = R // 128
    KT = n // 128
    NCH = n // ch
    FT = fdim // 128
    assert KT // ncores == S
    DCH = min(8, S)
    SPC = S // DCH
    assert S % DCH == 0
    HACC = 2 * NCH

    nc = bacc.Bacc(
        "TRN2", target_bir_lowering=False, debug=False, num_devices=ncores
    )
    A_d = nc.dram_tensor("A", [R, n], F32, kind="ExternalInput").ap()
    X_d = nc.dram_tensor("X", [R, fdim], F32, kind="ExternalInput").ap()
    W_d = nc.dram_tensor("W", [fdim, cdim], F32, kind="ExternalInput").ap()
    out_d = nc.dram_tensor("out", [R, cdim], F32, kind="ExternalOutput").ap()
    wu_in_d = nc.dram_tensor("wu_in", [128], F32).ap()
    wu_out_d = nc.dram_tensor("wu_out", [ncores * 128], F32, addr_space="Shared").ap()
    ys_in_d = [
        nc.dram_tensor(f"ys_in{c}", [SPC * 128, cdim], BF16).ap()
        for c in range(DCH)
    ]
    ys_out_d = [
        nc.dram_tensor(
            f"ys_out{c}", [ncores * SPC * 128, cdim], BF16, addr_space="Shared"
        ).ap()
        for c in range(DCH)
    ]
    groups = [list(range(ncores))]

    with tile.TileContext(nc) as tc, ExitStack() as ctx:
        const_pool = ctx.enter_context(tc.tile_pool(name="const", bufs=1))
        ident = const_pool.tile([128, 128], BF16)
        make_identity(nc, ident[:])

        at_pool = ctx.enter_context(tc.tile_pool(name="atp", bufs=1))
        AT = at_pool.tile([128, S * KT * 128], BF16)
        ys_pool = ctx.enter_context(tc.tile_pool(name="ysp", bufs=1))
        ys_sb = ys_pool.tile([128, KT * cdim], BF16)

        small_pool = ctx.enter_context(tc.tile_pool(name="small", bufs=1))
        xw_f32 = small_pool.tile([128, S * cdim], F32)   # XW then Ys_loc in place
        acc_sb = small_pool.tile([128, S * cdim], F32)
        ysch_bf = small_pool.tile([128, SPC * cdim], BF16)
        Dacc = small_pool.tile([128, S * HACC], F32)
        Dsum = small_pool.tile([128, S], F32)
        Dsq = small_pool.tile([128, S], F32)
        d_loc = small_pool.tile([128, S], F32)
        bias1 = small_pool.tile([128, 1], F32)
        nc.gpsimd.memset(bias1[:], 1.0 + 1e-10)

        # Dep-free warmup collective: starts the ncfw/NCCL init immediately.
        nc.gpsimd.collective_compute(
            "AllGather",
            mybir.AluOpType.bypass,
            replica_groups=groups,
            ins=[wu_in_d],
            outs=[wu_out_d],
        )

        # ---- Phase 0: XW_loc = X_shard @ W (bf16) ----
        with tc.tile_pool(name="ph0", bufs=2) as ph0, \
             tc.tile_pool(name="ph0c", bufs=1) as ph0c, \
             tc.tile_pool(name="ph0ps", bufs=2, space="PSUM") as ph0ps:
            w_f32 = ph0c.tile([128, FT * cdim], F32)
            w_bf = ph0c.tile([128, FT * cdim], BF16)
            for f in range(FT):
                nc.sync.dma_start(
                    w_f32[:, f * cdim:(f + 1) * cdim],
                    W_d[f * 128:(f + 1) * 128, :],
                )
            nc.vector.tensor_copy(w_bf[:], w_f32[:])
            xT = ph0c.tile([128, S * FT * 128], BF16)
            for s in range(S):
                x_f32 = ph0.tile([128, fdim], F32)
                nc.sync.dma_start(x_f32[:], X_d[s * 128:(s + 1) * 128, :])
                x_bf = ph0.tile([128, fdim], BF16)
                nc.vector.tensor_copy(x_bf[:], x_f32[:])
                pxt = ph0ps.tile([128, fdim], BF16)
                for f in range(FT):
                    nc.tensor.transpose(
                        pxt[:, f * 128:(f + 1) * 128],
                        x_bf[:, f * 128:(f + 1) * 128],
                        ident[:],
                    )
                nc.scalar.copy(xT[:, (s * FT) * 128:(s * FT + FT) * 128], pxt[:])
            for s in range(S):
                pxw = ph0ps.tile([128, cdim], F32)
                for f in range(FT):
                    nc.tensor.matmul(
                        pxw[:],
                        lhsT=xT[:, (s * FT + f) * 128:(s * FT + f + 1) * 128],
                        rhs=w_bf[:, f * cdim:(f + 1) * cdim],
                        start=(f == 0),
                        stop=(f == FT - 1),
                    )
                nc.vector.tensor_copy(xw_f32[:, s * cdim:(s + 1) * cdim], pxw[:])

        # ---- Phase 1 with interleaved bursts + chunked Ys AllGathers ----
        TPC = ch // 128
        ready_kts: list = []
        burst_n = [0] * S
        pending = []

        with tc.tile_pool(name="ast", bufs=4) as ast, \
             tc.tile_pool(name="abf", bufs=3) as abf, \
             tc.tile_pool(name="bur", bufs=3, space="PSUM") as bur, \
             tc.tile_pool(name="tps", bufs=3, space="PSUM") as tps, \
             tc.tile_pool(name="outp", bufs=2) as outp:

            def emit_burst(r, kts):
                if not kts:
                    return
                pb = bur.tile([128, cdim], F32, tag="burst")
                for idx, kt in enumerate(kts):
                    nc.tensor.matmul(
                        pb[:],
                        lhsT=AT[:, (r * KT + kt) * 128:(r * KT + kt + 1) * 128],
                        rhs=ys_sb[:, kt * cdim:(kt + 1) * cdim],
                        start=(idx == 0),
                        stop=(idx == len(kts) - 1),
                    )
                dst = acc_sb[:, r * cdim:(r + 1) * cdim]
                if burst_n[r] == 0:
                    nc.vector.tensor_copy(dst, pb[:])
                else:
                    nc.vector.tensor_add(dst, dst, pb[:])
                burst_n[r] += 1

            def emit_post(dc, s_hi):
                """DMA the gathered Ys chunk into ys_sb and burst it."""
                new_kts = []
                for j in range(ncores):
                    kt0 = j * S + dc * SPC
                    new_kts.extend(range(kt0, kt0 + SPC))
                    nc.gpsimd.dma_start(
                        ys_sb[:, kt0 * cdim:(kt0 + SPC) * cdim].rearrange(
                            "p (k c) -> p k c", k=SPC
                        ),
                        ys_out_d[dc][
                            j * SPC * 128:(j + 1) * SPC * 128, :
                        ].rearrange("(k p) c -> p k c", p=128),
                    )
                for r in range(s_hi + 1):
                    emit_burst(r, sorted(new_kts))
                ready_kts.extend(new_kts)

            for s in range(S):
                for c in range(NCH):
                    a_ch = ast.tile([128, ch], F32)
                    nc.sync.dma_start(
                        a_ch[:], A_d[s * 128:(s + 1) * 128, c * ch:(c + 1) * ch]
                    )
                    a_bf = abf.tile([128, ch], BF16)
                    i = s * NCH + c
                    H = ch // 2
                    nc.scalar.activation(
                        a_bf[:, :H], a_ch[:, :H], AF.Copy,
                        accum_out=Dacc[:, 2 * i:2 * i + 1],
                    )
                    nc.vector.tensor_scalar(
                        a_bf[:, H:], a_ch[:, H:], 1.0, 0.0,
                        mybir.AluOpType.mult,
                        mybir.AluOpType.add,
                        accum_out=Dacc[:, 2 * i + 1:2 * i + 2],
                    )
                    pt = tps.tile([128, TPC * 128], BF16)
                    for t in range(TPC):
                        nc.tensor.transpose(
                            pt[:, t * 128:(t + 1) * 128],
                            a_bf[:, t * 128:(t + 1) * 128],
                            ident[:],
                        )
                    kt0 = c * TPC
                    dst = AT[:, (s * KT + kt0) * 128:(s * KT + kt0 + TPC) * 128]
                    if i % 2 == 0:
                        nc.vector.tensor_copy(dst, pt[:])
                    else:
                        nc.scalar.copy(dst, pt[:])

                if (s + 1) % SPC == 0:
                    # d/Ys pipeline first: it feeds the AllGather trigger,
                    # which is the critical path; bursts below are slack.
                    dc = s // SPC
                    s0 = dc * SPC
                    nc.vector.tensor_reduce(
                        Dsum[:, s0:s0 + SPC],
                        Dacc[:, s0 * HACC:(s0 + SPC) * HACC].rearrange(
                            "p (s c) -> p s c", s=SPC
                        ),
                        axis=mybir.AxisListType.X,
                        op=mybir.AluOpType.add,
                    )
                    nc.scalar.activation(
                        Dsq[:, s0:s0 + SPC], Dsum[:, s0:s0 + SPC],
                        AF.Sqrt, bias=bias1[:],
                    )
                    nc.vector.reciprocal(
                        d_loc[:, s0:s0 + SPC], Dsq[:, s0:s0 + SPC]
                    )
                    for sl in range(s0, s0 + SPC):
                        nc.vector.tensor_scalar_mul(
                            xw_f32[:, sl * cdim:(sl + 1) * cdim],
                            xw_f32[:, sl * cdim:(sl + 1) * cdim],
                            d_loc[:, sl:sl + 1],
                        )
                    nc.vector.tensor_copy(
                        ysch_bf[:], xw_f32[:, s0 * cdim:(s0 + SPC) * cdim]
                    )
                    nc.gpsimd.dma_start(
                        ys_in_d[dc].rearrange("(k p) c -> p k c", p=128),
                        ysch_bf[:].rearrange("p (k c) -> p k c", k=SPC),
                    )
                    nc.gpsimd.collective_compute(
                        "AllGather",
                        mybir.AluOpType.bypass,
                        replica_groups=groups,
                        ins=[ys_in_d[dc]],
                        outs=[ys_out_d[dc]],
                    )
                    pending.append((dc, s + delay))

                emit_burst(s, ready_kts)

                while pending and pending[0][1] <= s:
                    dc, _ = pending.pop(0)
                    emit_post(dc, s)

            # Tail: posts whose scheduled stripe lies past the loop, in
            # chunk order -- each pipelines with its AllGather completing.
            for dc, _ in pending:
                emit_post(dc, S - 1)
            pending = []

            # ---- Final: out[r] = relu(d * (acc + Ys_loc[r])) ----
            for r in range(S):
                assert burst_n[r] > 0, f"no bursts for stripe {r}"
                dst = acc_sb[:, r * cdim:(r + 1) * cdim]
                nc.vector.tensor_add(dst, dst, xw_f32[:, r * cdim:(r + 1) * cdim])
                ot = outp.tile([128, cdim], F32)
                nc.scalar.activation(ot[:], dst, AF.Relu, scale=d_loc[:, r:r + 1])
                nc.scalar.dma_start(out_d[r * 128:(r + 1) * 128, :], ot[:])

    nc.compile()
    return nc


def build_v6f(n=N, fdim=FDIM, cdim=CDIM, ncores=NCORES, ch=1024, delay=None,
              dch=None):
    """V6f: v6 + dead-zone fixes (warmup-first, 2 bounce bufs, scalar bounces).

    As each pair of stripes finishes streaming, its rowsums are complete, so
    d (and Ys = d*XW) for those rows is computable LOCALLY. AllGather the
    scaled bf16 Ys chunk directly -- no XW gather, no d exchange, no
    post-collective transposes or per-tile rescales. Main-matmul 'bursts'
    interleave with the streaming/transpose pipeline as chunks land.
    """
    R = n // ncores
    S = R // 128
    KT = n // 128
    NCH = n // ch
    FT = fdim // 128
    assert KT // ncores == S
    if delay is None:
        delay = int(os.environ.get("GCN_DELAY", "5"))
    if dch is None:
        dch = int(os.environ.get("GCN_DCH", "8"))
    DCH = min(dch, S)
    SPC = S // DCH
    assert S % DCH == 0
    HACC = 2 * NCH

    nc = bacc.Bacc(
        "TRN2", target_bir_lowering=False, debug=False, num_devices=ncores
    )
    A_d = nc.dram_tensor("A", [R, n], F32, kind="ExternalInput").ap()
    X_d = nc.dram_tensor("X", [R, fdim], F32, kind="ExternalInput").ap()
    W_d = nc.dram_tensor("W", [fdim, cdim], F32, kind="ExternalInput").ap()
    out_d = nc.dram_tensor("out", [R, cdim], F32, kind="ExternalOutput").ap()
    wu_in_d = nc.dram_tensor("wu_in", [128], F32).ap()
    wu_out_d = nc.dram_tensor("wu_out", [ncores * 128], F32, addr_space="Shared").ap()
    ys_in_d = [
        nc.dram_tensor(f"ys_in{c}", [SPC * 128, cdim], BF16).ap()
        for c in range(DCH)
    ]
    ys_out_d = [
        nc.dram_tensor(
            f"ys_out{c}", [ncores * SPC * 128, cdim], BF16, addr_space="Shared"
        ).ap()
        for c in range(DCH)
    ]
    groups = [list(range(ncores))]

    with tile.TileContext(nc) as tc, ExitStack() as ctx:
        # Warmup doorbell first: pulls the runtime CC barrier earlier.
        # GCN_WU=0 skips it: the first real AG (trigger ~16us) rings the
        # doorbell almost as early and saves ~13us of CC-stream time.
        if os.environ.get("GCN_WU", "1") == "1":
            nc.gpsimd.collective_compute(
                "AllGather",
                mybir.AluOpType.bypass,
                replica_groups=groups,
                ins=[wu_in_d],
                outs=[wu_out_d],
            )
        const_pool = ctx.enter_context(tc.tile_pool(name="const", bufs=1))
        ident = const_pool.tile([128, 128], BF16)
        make_identity(nc, ident[:])

        at_pool = ctx.enter_context(tc.tile_pool(name="atp", bufs=1))
        AT = at_pool.tile([128, S * KT * 128], BF16)
        ys_pool = ctx.enter_context(tc.tile_pool(name="ysp", bufs=1))
        ys_sb = ys_pool.tile([128, KT * cdim], BF16)

        small_pool = ctx.enter_context(tc.tile_pool(name="small", bufs=1))
        xw_f32 = small_pool.tile([128, S * cdim], F32)   # XW then Ys_loc in place
        acc_sb = small_pool.tile([128, S * cdim], F32)
        Dacc = small_pool.tile([128, S * HACC], F32)
        Dsum = small_pool.tile([128, S], F32)
        Dsq = small_pool.tile([128, S], F32)
        d_loc = small_pool.tile([128, S], F32)
        bias1 = small_pool.tile([128, 1], F32)
        nc.gpsimd.memset(bias1[:], 1.0 + 1e-10)
        yschp = ctx.enter_context(tc.tile_pool(name="ysch", bufs=2))

        # ---- Phase 0: XW_loc = X_shard @ W (bf16) ----
        # GCN_XWQ=scalar moves the X/W input loads off the sync queue so
        # the A stream starts at t~1us instead of ~12us (left-shifts the
        # stripe ends, triggers, and tail by ~10us). Zero-dep enqueues at
        # t=0 cannot head-of-line block anything.
        xwq = nc.scalar if os.environ.get("GCN_XWQ", "scalar") == "scalar" else nc.sync
        with tc.tile_pool(name="ph0", bufs=2) as ph0, \
             tc.tile_pool(name="ph0c", bufs=1) as ph0c, \
             tc.tile_pool(name="ph0ps", bufs=2, space="PSUM") as ph0ps:
            w_f32 = ph0c.tile([128, FT * cdim], F32)
            w_bf = ph0c.tile([128, FT * cdim], BF16)
            for f in range(FT):
                xwq.dma_start(
                    w_f32[:, f * cdim:(f + 1) * cdim],
                    W_d[f * 128:(f + 1) * 128, :],
                )
            nc.vector.tensor_copy(w_bf[:], w_f32[:])
            xT = ph0c.tile([128, S * FT * 128], BF16)
            for s in range(S):
                x_f32 = ph0.tile([128, fdim], F32)
                xwq.dma_start(x_f32[:], X_d[s * 128:(s + 1) * 128, :])
                x_bf = ph0.tile([128, fdim], BF16)
                nc.vector.tensor_copy(x_bf[:], x_f32[:])
                pxt = ph0ps.tile([128, fdim], BF16)
                for f in range(FT):
                    nc.tensor.transpose(
                        pxt[:, f * 128:(f + 1) * 128],
                        x_bf[:, f * 128:(f + 1) * 128],
                        ident[:],
                    )
                nc.scalar.copy(xT[:, (s * FT) * 128:(s * FT + FT) * 128], pxt[:])
            for s in range(S):
                pxw = ph0ps.tile([128, cdim], F32)
                for f in range(FT):
                    nc.tensor.matmul(
                        pxw[:],
                        lhsT=xT[:, (s * FT + f) * 128:(s * FT + f + 1) * 128],
                        rhs=w_bf[:, f * cdim:(f + 1) * cdim],
                        start=(f == 0),
                        stop=(f == FT - 1),
                    )
                nc.vector.tensor_copy(xw_f32[:, s * cdim:(s + 1) * cdim], pxw[:])

        # ---- Phase 1 with interleaved bursts + chunked Ys AllGathers ----
        TPC = ch // 128
        ready_kts: list = []
        burst_n = [0] * S
        pending = []

        with tc.tile_pool(name="ast", bufs=int(os.environ.get("GCN_AST", "4"))) as ast, \
             tc.tile_pool(name="abf", bufs=int(os.environ.get("GCN_ABF", "3"))) as abf, \
             tc.tile_pool(name="bur", bufs=3, space="PSUM") as bur, \
             tc.tile_pool(name="tps", bufs=3, space="PSUM") as tps, \
             tc.tile_pool(name="outp", bufs=2) as outp:

            def emit_burst(r, kts):
                if not kts:
                    return
                pb = bur.tile([128, cdim], F32, tag="burst")
                for idx, kt in enumerate(kts):
                    nc.tensor.matmul(
                        pb[:],
                        lhsT=AT[:, (r * KT + kt) * 128:(r * KT + kt + 1) * 128],
                        rhs=ys_sb[:, kt * cdim:(kt + 1) * cdim],
                        start=(idx == 0),
                        stop=(idx == len(kts) - 1),
                    )
                dst = acc_sb[:, r * cdim:(r + 1) * cdim]
                if burst_n[r] == 0:
                    nc.vector.tensor_copy(dst, pb[:])
                else:
                    nc.vector.tensor_add(dst, dst, pb[:])
                burst_n[r] += 1

            def emit_post(dc, s_hi, eng=None, fin=False):
                """DMA the gathered Ys chunk into ys_sb and burst it.
                Mid-stream gathers ride gpsimd (self-timed); tail gathers
                ride the then-idle sync HWDGE (3x faster). fin=True
                interleaves each stripe's final relu/store right after its
                last burst so the tail pipelines instead of serializing."""
                if eng is None:
                    eng = nc.gpsimd
                new_kts = []
                for j in range(ncores):
                    kt0 = j * S + dc * SPC
                    new_kts.extend(range(kt0, kt0 + SPC))
                    eng.dma_start(
                        ys_sb[:, kt0 * cdim:(kt0 + SPC) * cdim].rearrange(
                            "p (k c) -> p k c", k=SPC
                        ),
                        ys_out_d[dc][
                            j * SPC * 128:(j + 1) * SPC * 128, :
                        ].rearrange("(k p) c -> p k c", p=128),
                    )
                for r in range(s_hi + 1):
                    emit_burst(r, sorted(new_kts))
                    if fin:
                        emit_final(r)
                ready_kts.extend(new_kts)

            def emit_final(r):
                assert burst_n[r] > 0, f"no bursts for stripe {r}"
                dst = acc_sb[:, r * cdim:(r + 1) * cdim]
                nc.vector.tensor_add(dst, dst, xw_f32[:, r * cdim:(r + 1) * cdim])
                ot = outp.tile([128, cdim], F32)
                nc.scalar.activation(ot[:], dst, AF.Relu, scale=d_loc[:, r:r + 1])
                nc.scalar.dma_start(out_d[r * 128:(r + 1) * 128, :], ot[:])

            for s in range(S):
                for c in range(NCH):
                    a_ch = ast.tile([128, ch], F32)
                    nc.sync.dma_start(
                        a_ch[:], A_d[s * 128:(s + 1) * 128, c * ch:(c + 1) * ch]
                    )
                    a_bf = abf.tile([128, ch], BF16)
                    i = s * NCH + c
                    H = ch // 2
                    nc.scalar.activation(
                        a_bf[:, :H], a_ch[:, :H], AF.Copy,
                        accum_out=Dacc[:, 2 * i:2 * i + 1],
                    )
                    nc.vector.tensor_scalar(
                        a_bf[:, H:], a_ch[:, H:], 1.0, 0.0,
                        mybir.AluOpType.mult,
                        mybir.AluOpType.add,
                        accum_out=Dacc[:, 2 * i + 1:2 * i + 2],
                    )
                    pt = tps.tile([128, TPC * 128], BF16)
                    for t in range(TPC):
                        nc.tensor.transpose(
                            pt[:, t * 128:(t + 1) * 128],
                            a_bf[:, t * 128:(t + 1) * 128],
                            ident[:],
                        )
                    kt0 = c * TPC
                    dst = AT[:, (s * KT + kt0) * 128:(s * KT + kt0 + TPC) * 128]
                    if i % 2 == 0:
                        nc.vector.tensor_copy(dst, pt[:])
                    else:
                        nc.scalar.copy(dst, pt[:])

                if (s + 1) % SPC == 0:
                    # d/Ys pipeline first: it feeds the AllGather trigger,
                    # which is the critical path; bursts below are slack.
                    dc = s // SPC
                    s0 = dc * SPC
                    nc.vector.tensor_reduce(
                        Dsum[:, s0:s0 + SPC],
                        Dacc[:, s0 * HACC:(s0 + SPC) * HACC].rearrange(
                            "p (s c) -> p s c", s=SPC
                        ),
                        axis=mybir.AxisListType.X,
                        op=mybir.AluOpType.add,
                    )
                    nc.scalar.activation(
                        Dsq[:, s0:s0 + SPC], Dsum[:, s0:s0 + SPC],
                        AF.Sqrt, bias=bias1[:],
                    )
                    nc.vector.reciprocal(
                        d_loc[:, s0:s0 + SPC], Dsq[:, s0:s0 + SPC]
                    )
                    for sl in range(s0, s0 + SPC):
                        nc.vector.tensor_scalar_mul(
                            xw_f32[:, sl * cdim:(sl + 1) * cdim],
                            xw_f32[:, sl * cdim:(sl + 1) * cdim],
                            d_loc[:, sl:sl + 1],
                        )
                    ysch_bf = yschp.tile([128, SPC * cdim], BF16)
                    nc.vector.tensor_copy(
                        ysch_bf[:], xw_f32[:, s0 * cdim:(s0 + SPC) * cdim]
                    )
                    nc.scalar.dma_start(
                        ys_in_d[dc].rearrange("(k p) c -> p k c", p=128),
                        ysch_bf[:].rearrange("p (k c) -> p k c", k=SPC),
                    )
                    nc.gpsimd.collective_compute(
                        "AllGather",
                        mybir.AluOpType.bypass,
                        replica_groups=groups,
                        ins=[ys_in_d[dc]],
                        outs=[ys_out_d[dc]],
                    )
                    pending.append((dc, s + delay))

                emit_burst(s, ready_kts)

                while pending and pending[0][1] <= s:
                    dc, _ = pending.pop(0)
                    emit_post(dc, s)

            # Tail: posts whose scheduled stripe lies past the loop, in
            # chunk order -- each pipelines with its AllGather completing.
            # The last post interleaves the per-stripe finals.
            tailopt = os.environ.get("GCN_TAILOPT", "1") == "1"
            tail = [dc for dc, _ in pending]
            pending = []
            for idx, dc in enumerate(tail):
                emit_post(
                    dc, S - 1,
                    eng=(nc.sync if tailopt else None),
                    fin=(tailopt and idx == len(tail) - 1),
                )
            if not (tailopt and tail):
                for r in range(S):
                    emit_final(r)

    nc.compile()
    return nc



def build_v8(n=N, fdim=FDIM, cdim=CDIM, ncores=NCORES, ch=None, chunks=None,
             delay=None, warmup=None):
    """V8r2: v6 with the serialization stalls removed.

    Trace findings driving this design (v6 = 278us):
      - A-stream stalled mid-kernel: single ysch bounce buffer + bounce DMAs
        and collective triggers sharing the gpsimd queue head-of-line blocked
        the DVE cast pipeline, backpressuring the stream.
      - Collectives serialize on one CC stream behind a ~45-62us runtime
        barrier whose end tracks the first collective doorbell; +~11us
        first-op setup. AllGathers cost ~10us latency + transfer at
        ~20-35us/MB while contending with the A-stream for HBM.
      - Main matmuls issue at 132ns (512 = 68us), all gated on gathered Ys.

    Fixes:
      - X/W loads on the scalar HWDGE queue; streaming pools allocated
        BEFORE phase0 pools so the first A-chunk DMA has no SBUF aliasing
        dependency on phase0 -> stream starts at t~1us.
      - Variable AllGather chunking (stripes per chunk, default [1,2,2,3]):
        tiny first chunk rings the CC doorbell early (pulls the barrier end
        forward) and starts the mains early; later chunks amortize latency.
      - Bounce DMAs on scalar queue; gather-backs on the sync queue
        (self-timed behind the A stream); ONLY triggers on gpsimd.
      - Burst accumulators live in PSUM (4 banks); one start=True per bank
        (PSUM lazy-zero is 2KB-bank granular) and skip_group_check bursts.
    """
    if ch is None:
        ch = int(os.environ.get("GCN_CH", "1024"))
    if chunks is None:
        chunks = [int(x) for x in os.environ.get("GCN_CHUNKS", "2,3,2,1").split(",")]
    if delay is None:
        delay = os.environ.get("GCN_DUE", "6,99,99,99")
    due_list = [int(x) for x in str(delay).split(",")] if isinstance(delay, str) else None
    if warmup is None:
        warmup = os.environ.get("GCN_WU", "0") == "1"
    R = n // ncores
    S = R // 128
    KT = n // 128
    FT = fdim // 128
    ch = min(ch, n)
    NCH = n // ch            # chunks per stripe
    TPC = ch // 128          # transposes per chunk
    GRP = min(8, TPC)        # transposes per PSUM drain group
    NG = TPC // GRP
    assert TPC % GRP == 0
    HACC = NCH
    if sum(chunks) != S:     # fall back to one chunk per stripe group
        chunks = [1] * S
    DCH = len(chunks)
    cs0 = [sum(chunks[:i]) for i in range(DCH)]   # first stripe of chunk

    nc = bacc.Bacc(
        "TRN2", target_bir_lowering=False, debug=False, num_devices=ncores
    )
    A_d = nc.dram_tensor("A", [R, n], F32, kind="ExternalInput").ap()
    X_d = nc.dram_tensor("X", [R, fdim], F32, kind="ExternalInput").ap()
    W_d = nc.dram_tensor("W", [fdim, cdim], F32, kind="ExternalInput").ap()
    out_d = nc.dram_tensor("out", [R, cdim], F32, kind="ExternalOutput").ap()
    if warmup:
        wu_in_d = nc.dram_tensor("wu_in", [128], F32).ap()
        wu_out_d = nc.dram_tensor(
            "wu_out", [ncores * 128], F32, addr_space="Shared"
        ).ap()
    ys_in_d = [
        nc.dram_tensor(f"ys_in{c}", [chunks[c] * 128, cdim], BF16).ap()
        for c in range(DCH)
    ]
    ys_out_d = [
        nc.dram_tensor(
            f"ys_out{c}", [ncores * chunks[c] * 128, cdim], BF16,
            addr_space="Shared",
        ).ap()
        for c in range(DCH)
    ]
    groups = [list(range(ncores))]

    with tile.TileContext(nc) as tc, ExitStack() as ctx:
        # Dep-free warmup collective FIRST: rings the CC doorbell at t~0.
        if warmup:
            nc.gpsimd.collective_compute(
                "AllGather",
                mybir.AluOpType.bypass,
                replica_groups=groups,
                ins=[wu_in_d],
                outs=[wu_out_d],
            )

        const_pool = ctx.enter_context(tc.tile_pool(name="const", bufs=1))
        ident = const_pool.tile([128, 128], BF16)
        make_identity(nc, ident[:])

        at_pool = ctx.enter_context(tc.tile_pool(name="atp", bufs=1))
        AT = at_pool.tile([128, S * KT * 128], BF16)
        ys_pool = ctx.enter_context(tc.tile_pool(name="ysp", bufs=1))
        ys_sb = ys_pool.tile([128, KT * cdim], BF16)

        small_pool = ctx.enter_context(tc.tile_pool(name="small", bufs=1))
        xw_f32 = small_pool.tile([128, S * cdim], F32)   # XW then Ys_loc
        Dacc = small_pool.tile([128, S * HACC], F32)
        Dsum = small_pool.tile([128, S], F32)
        Dsq = small_pool.tile([128, S], F32)
        d_loc = small_pool.tile([128, S], F32)
        bias1 = small_pool.tile([128, 1], F32)
        nc.gpsimd.memset(bias1[:], 1.0 + 1e-10)

        # Streaming pools allocated BEFORE phase0 pools: the first A-chunk
        # DMA must not alias phase0 SBUF (that dependency delayed the
        # stream start to 38us in the first v8 cut).
        ast = ctx.enter_context(tc.tile_pool(name="ast", bufs=2))
        abf = ctx.enter_context(tc.tile_pool(name="abf", bufs=2))
        yschp = ctx.enter_context(tc.tile_pool(name="ysch", bufs=2))
        outp = ctx.enter_context(tc.tile_pool(name="outp", bufs=2))

        # ---- Phase 0: XW_loc = X_shard @ W; loads on scalar queue; small
        # footprint (per-stripe transpose+matmul, no big xT buffer) ----
        with tc.tile_pool(name="ph0", bufs=2) as ph0, \
             tc.tile_pool(name="ph0c", bufs=1) as ph0c, \
             tc.tile_pool(name="ph0x", bufs=2) as ph0x, \
             tc.tile_pool(name="ph0ps", bufs=2, space="PSUM") as ph0ps:
            w_f32 = ph0c.tile([128, FT * cdim], F32)
            w_bf = ph0c.tile([128, FT * cdim], BF16)
            for f in range(FT):
                nc.scalar.dma_start(
                    w_f32[:, f * cdim:(f + 1) * cdim],
                    W_d[f * 128:(f + 1) * 128, :],
                )
            nc.vector.tensor_copy(w_bf[:], w_f32[:])
            for s in range(S):
                x_f32 = ph0.tile([128, fdim], F32)
                nc.scalar.dma_start(x_f32[:], X_d[s * 128:(s + 1) * 128, :])
                x_bf = ph0.tile([128, fdim], BF16)
                nc.vector.tensor_copy(x_bf[:], x_f32[:])
                pxt = ph0ps.tile([128, fdim], BF16)
                for f in range(FT):
                    nc.tensor.transpose(
                        pxt[:, f * 128:(f + 1) * 128],
                        x_bf[:, f * 128:(f + 1) * 128],
                        ident[:],
                    )
                xTs = ph0x.tile([128, FT * 128], BF16)
                nc.scalar.copy(xTs[:], pxt[:])
                pxw = ph0ps.tile([128, cdim], F32)
                for f in range(FT):
                    nc.tensor.matmul(
                        pxw[:],
                        lhsT=xTs[:, f * 128:(f + 1) * 128],
                        rhs=w_bf[:, f * cdim:(f + 1) * cdim],
                        start=(f == 0),
                        stop=(f == FT - 1),
                    )
                nc.vector.tensor_copy(xw_f32[:, s * cdim:(s + 1) * cdim], pxw[:])

        # ---- Phase 1: stream A + interleaved bursts + chunked Ys AGs ----
        ready_kts: list = []
        started = [False] * S
        pending = []

        with tc.tile_pool(name="burp", bufs=1, space="PSUM") as burp, \
             tc.tile_pool(name="tps", bufs=3, space="PSUM") as tps:
            # Persistent PSUM accumulators: one [128, cdim] f32 slice per
            # stripe; bursts accumulate via matmul start=False.
            # PSUM lazy-zero is zero-region (2KB bank) granular: start=True
            # marks the WHOLE bank pending-zero, so with two stripes packed
            # per bank only the bank's FIRST-EVER matmul may use start=True.
            # The partner stripe's first touch hits still-pending bytes and
            # overwrites (= implicit init); later bursts accumulate. Burst
            # emission is ascending in r, so the even partner starts first.
            bur = burp.tile([128, S * cdim], F32)
            spb = max(1, 2048 // (cdim * 4))     # stripes per psum bank
            bank_started = [False] * ((S + spb - 1) // spb)

            def emit_burst(r, kts):
                if not kts:
                    return
                first = not bank_started[r // spb]
                bank_started[r // spb] = True
                started[r] = True
                for idx, kt in enumerate(kts):
                    nc.tensor.matmul(
                        bur[:, r * cdim:(r + 1) * cdim],
                        lhsT=AT[:, (r * KT + kt) * 128:(r * KT + kt + 1) * 128],
                        rhs=ys_sb[:, kt * cdim:(kt + 1) * cdim],
                        start=(first and idx == 0),
                        stop=(idx == len(kts) - 1),
                        skip_group_check=True,
                    )

            def emit_post(dc, s_hi, eng=None):
                """Gather-back DMAs + bursts of the newly-landed kts.
                Mid-stream gathers ride gpsimd (idle => self-timed); tail
                gathers ride the then-idle sync HWDGE (fast)."""
                if eng is None:
                    eng = nc.gpsimd
                spc = chunks[dc]
                new_kts = []
                for j in range(ncores):
                    kt0 = j * S + cs0[dc]
                    new_kts.extend(range(kt0, kt0 + spc))
                    eng.dma_start(
                        ys_sb[:, kt0 * cdim:(kt0 + spc) * cdim].rearrange(
                            "p (k c) -> p k c", k=spc
                        ),
                        ys_out_d[dc][
                            j * spc * 128:(j + 1) * spc * 128, :
                        ].rearrange("(k p) c -> p k c", p=128),
                    )
                for r in range(s_hi + 1):
                    emit_burst(r, sorted(new_kts))
                ready_kts.extend(new_kts)

            dc_next = 0
            for s in range(S):
                for c in range(NCH):
                    a_ch = ast.tile([128, ch], F32)
                    nc.sync.dma_start(
                        a_ch[:], A_d[s * 128:(s + 1) * 128, c * ch:(c + 1) * ch]
                    )
                    a_bf = abf.tile([128, ch], BF16)
                    i = s * NCH + c
                    # full-chunk casts on alternating engines: each engine
                    # gets two chunk-times of slack per cast, so per-chunk
                    # jitter does not throttle the ast-recycle semaphores
                    if i % 2 == 0:
                        nc.scalar.activation(
                            a_bf[:], a_ch[:], AF.Copy,
                            accum_out=Dacc[:, i:i + 1],
                        )
                    else:
                        nc.vector.tensor_scalar(
                            a_bf[:], a_ch[:], 1.0, 0.0,
                            mybir.AluOpType.mult,
                            mybir.AluOpType.add,
                            accum_out=Dacc[:, i:i + 1],
                        )
                    for g in range(NG):
                        pt = tps.tile([128, GRP * 128], BF16)
                        for t in range(GRP):
                            tt = g * GRP + t
                            nc.tensor.transpose(
                                pt[:, t * 128:(t + 1) * 128],
                                a_bf[:, tt * 128:(tt + 1) * 128],
                                ident[:],
                            )
                        kt0 = c * TPC + g * GRP
                        dst = AT[:, (s * KT + kt0) * 128:(s * KT + kt0 + GRP) * 128]
                        if (i * NG + g) % 2 == 0:
                            nc.vector.tensor_copy(dst, pt[:])
                        else:
                            nc.scalar.copy(dst, pt[:])

                if dc_next < DCH and s == cs0[dc_next] + chunks[dc_next] - 1:
                    # d/Ys pipeline first: it feeds the AllGather trigger.
                    dc = dc_next
                    dc_next += 1
                    s0 = cs0[dc]
                    spc = chunks[dc]
                    nc.vector.tensor_reduce(
                        Dsum[:, s0:s0 + spc],
                        Dacc[:, s0 * HACC:(s0 + spc) * HACC].rearrange(
                            "p (s c) -> p s c", s=spc
                        ),
                        axis=mybir.AxisListType.X,
                        op=mybir.AluOpType.add,
                    )
                    nc.scalar.activation(
                        Dsq[:, s0:s0 + spc], Dsum[:, s0:s0 + spc],
                        AF.Sqrt, bias=bias1[:],
                    )
                    nc.vector.reciprocal(
                        d_loc[:, s0:s0 + spc], Dsq[:, s0:s0 + spc]
                    )
                    for sl in range(s0, s0 + spc):
                        nc.vector.tensor_scalar_mul(
                            xw_f32[:, sl * cdim:(sl + 1) * cdim],
                            xw_f32[:, sl * cdim:(sl + 1) * cdim],
                            d_loc[:, sl:sl + 1],
                        )
                    ysch = yschp.tile([128, spc * cdim], BF16)
                    nc.vector.tensor_copy(
                        ysch[:], xw_f32[:, s0 * cdim:(s0 + spc) * cdim]
                    )
                    nc.gpsimd.dma_start(
                        ys_in_d[dc].rearrange("(k p) c -> p k c", p=128),
                        ysch[:].rearrange("p (k c) -> p k c", k=spc),
                    )
                    nc.gpsimd.collective_compute(
                        "AllGather",
                        mybir.AluOpType.bypass,
                        replica_groups=groups,
                        ins=[ys_in_d[dc]],
                        outs=[ys_out_d[dc]],
                    )
                    due = due_list[dc] if due_list and dc < len(due_list) else s + 6
                    pending.append((dc, due))

                emit_burst(s, ready_kts)

                while pending and pending[0][1] <= s:
                    dc, _ = pending.pop(0)
                    emit_post(dc, s)

            for dc, _ in pending:
                emit_post(dc, S - 1, eng=nc.sync)
            pending = []

            # ---- Final: out[r] = relu(d * (bur[r] + Ys_loc[r])) ----
            for r in range(S):
                assert started[r], f"no bursts for stripe {r}"
                dst = bur[:, r * cdim:(r + 1) * cdim]
                nc.vector.tensor_add(dst, dst, xw_f32[:, r * cdim:(r + 1) * cdim])
                ot = outp.tile([128, cdim], F32)
                nc.scalar.activation(ot[:], dst, AF.Relu, scale=d_loc[:, r:r + 1])
                nc.scalar.dma_start(out_d[r * 128:(r + 1) * 128, :], ot[:])

    nc.compile()
    return nc


def build_v9(n=N, fdim=FDIM, cdim=CDIM, ncores=NCORES, ch=None, chunks=None,
             due=None):
    """V9: replicate XW, exchange only d (32KB) instead of Ys (4MB).

    Trace-driven redesign. The v8 rounds showed the Ys AllGather chain is
    the wall: each AG costs ~10us latency + ~20-25us/MB on NeuronLink and
    they serialize on the one CC stream behind a ~40-60us runtime barrier,
    so moving 4MB of Ys costs ~120-140us regardless of chunking. Instead:

      - X^T (host-transposed layout, [F, N] f32) is replicated to every
        core; each core computes the FULL XW = X@W locally (lhsT = X^T
        tiles straight from DMA, zero transposes; +34us Tensor, +16MB HBM
        on the otherwise idle scalar queue).
      - Only d (1/sqrt degrees) is exchanged: 4 AllGathers of 4KB each
        (pure ~10us latency), triggered as each stripe-pair's rowsums
        complete during the A stream.
      - Ys tiles = dg-scaled XW tiles (in-place bf16); the GCN mains then
        run against SBUF-resident A^T exactly as before.

    Choreography laws learned from traces:
      - A DMA enqueue with an unmet dep blocks the issuing ENGINE, so the
        d bounce/AG/gather live on gpsimd only (idle => self-timed).
      - Ys scales are due-timed on DVE (emitted at the stripe position
        where DVE naturally arrives ~ when dg lands) to avoid stalling
        the cast pipeline.
      - PSUM lazy-zero is 2KB-bank granular: one start=True per bank.
    """
    if ch is None:
        ch = int(os.environ.get("GCN_CH", "1024"))
    if chunks is None:
        chunks = [int(x) for x in os.environ.get("GCN_CHUNKS", "4,4").split(",")]
    if due is None:
        due = os.environ.get("GCN_DUE", "6,99")
    due_list = [int(x) for x in str(due).split(",")]
    R = n // ncores
    S = R // 128
    KT = n // 128
    FT = fdim // 128
    ch = min(ch, n)
    NCH = n // ch
    TPC = ch // 128
    GRP = min(8, TPC)
    NG = TPC // GRP
    assert TPC % GRP == 0
    HACC = 2 * NCH
    if sum(chunks) != S:
        chunks = [1] * S
    DCH = len(chunks)
    cs0 = [sum(chunks[:i]) for i in range(DCH)]
    while len(due_list) < DCH:
        due_list.append(99)

    nc = bacc.Bacc(
        "TRN2", target_bir_lowering=False, debug=False, num_devices=ncores
    )
    A_d = nc.dram_tensor("A", [R, n], F32, kind="ExternalInput").ap()
    X_d = nc.dram_tensor("X", [R, fdim], F32, kind="ExternalInput").ap()
    XT_d = nc.dram_tensor("XT", [fdim, n], F32, kind="ExternalInput").ap()
    W_d = nc.dram_tensor("W", [fdim, cdim], F32, kind="ExternalInput").ap()
    out_d = nc.dram_tensor("out", [R, cdim], F32, kind="ExternalOutput").ap()
    d_in_d = [
        nc.dram_tensor(f"d_in{c}", [chunks[c] * 128], F32).ap()
        for c in range(DCH)
    ]
    d_out_d = [
        nc.dram_tensor(
            f"d_out{c}", [ncores * chunks[c] * 128], F32, addr_space="Shared"
        ).ap()
        for c in range(DCH)
    ]
    groups = [list(range(ncores))]

    with tile.TileContext(nc) as tc, ExitStack() as ctx:
        const_pool = ctx.enter_context(tc.tile_pool(name="const", bufs=1))
        ident = const_pool.tile([128, 128], BF16)
        make_identity(nc, ident[:])

        at_pool = ctx.enter_context(tc.tile_pool(name="atp", bufs=1))
        AT = at_pool.tile([128, S * KT * 128], BF16)
        ys_pool = ctx.enter_context(tc.tile_pool(name="ysp", bufs=1))
        ysb = ys_pool.tile([128, KT * cdim], BF16)   # XW_full then Ys in place

        small_pool = ctx.enter_context(tc.tile_pool(name="small", bufs=1))
        xw_f32 = small_pool.tile([128, S * cdim], F32)   # XW_loc then Ys_loc
        w_bf = small_pool.tile([128, FT * cdim], BF16)
        Dacc = small_pool.tile([128, S * HACC], F32)
        Dsum = small_pool.tile([128, S], F32)
        Dsq = small_pool.tile([128, S], F32)
        d_loc = small_pool.tile([128, S], F32)
        dg = small_pool.tile([128, KT], F32)    # gathered d, chunk-major cols
        bias1 = small_pool.tile([128, 1], F32)
        nc.gpsimd.memset(bias1[:], 1.0 + 1e-10)

        # Streaming pools before phase0 pools (no SBUF aliasing with ph0).
        ast = ctx.enter_context(tc.tile_pool(name="ast", bufs=2))
        abf = ctx.enter_context(tc.tile_pool(name="abf", bufs=2))
        xts = ctx.enter_context(tc.tile_pool(name="xts", bufs=1))
        xtb = ctx.enter_context(tc.tile_pool(name="xtb", bufs=2))
        outp = ctx.enter_context(tc.tile_pool(name="outp", bufs=2))
        burp = ctx.enter_context(tc.tile_pool(name="burp", bufs=1, space="PSUM"))
        tps = ctx.enter_context(tc.tile_pool(name="tps", bufs=2, space="PSUM"))
        xwps = ctx.enter_context(tc.tile_pool(name="xwps", bufs=2, space="PSUM"))

        # W load + cast (scalar queue); borrows an xts slot transiently.
        w_f32 = xts.tile([128, FT * cdim], F32, tag="xt_f")
        for f in range(FT):
            nc.scalar.dma_start(
                w_f32[:, f * cdim:(f + 1) * cdim], W_d[f * 128:(f + 1) * 128, :]
            )
        nc.vector.tensor_copy(w_bf[:], w_f32[:])

        # ---- Phase 0: XW_loc = X_shard @ W (for the +I diagonal term) ----
        with tc.tile_pool(name="ph0", bufs=2) as ph0:
            for s in range(S):
                x_f32 = ph0.tile([128, fdim], F32)
                nc.scalar.dma_start(x_f32[:], X_d[s * 128:(s + 1) * 128, :])
                x_bf = ph0.tile([128, fdim], BF16)
                nc.vector.tensor_copy(x_bf[:], x_f32[:])
                pxt = tps.tile([128, GRP * 128], BF16, tag="pt")
                for f in range(FT):
                    nc.tensor.transpose(
                        pxt[:, f * 128:(f + 1) * 128],
                        x_bf[:, f * 128:(f + 1) * 128],
                        ident[:],
                    )
                xTs = xtb.tile([128, FT * 128], BF16, tag="xt_b")
                nc.scalar.copy(xTs[:], pxt[:, :FT * 128])
                pxw = xwps.tile([128, cdim], F32)
                for f in range(FT):
                    nc.tensor.matmul(
                        pxw[:],
                        lhsT=xTs[:, f * 128:(f + 1) * 128],
                        rhs=w_bf[:, f * cdim:(f + 1) * cdim],
                        start=(f == 0),
                        stop=(f == FT - 1),
                    )
                nc.vector.tensor_copy(xw_f32[:, s * cdim:(s + 1) * cdim], pxw[:])

        # ---- Phase 1 ----
        ready_kts: list = []
        started = [False] * S
        pending = []

        bur = burp.tile([128, S * cdim], F32)
        spb = max(1, 2048 // (cdim * 4))
        bank_started = [False] * ((S + spb - 1) // spb)

        def emit_burst(r, kts):
            if not kts:
                return
            first = not bank_started[r // spb]
            bank_started[r // spb] = True
            started[r] = True
            for idx, kt in enumerate(kts):
                nc.tensor.matmul(
                    bur[:, r * cdim:(r + 1) * cdim],
                    lhsT=AT[:, (r * KT + kt) * 128:(r * KT + kt + 1) * 128],
                    rhs=ysb[:, kt * cdim:(kt + 1) * cdim],
                    start=(first and idx == 0),
                    stop=(idx == len(kts) - 1),
                    skip_group_check=True,
                )

        # XW units: one per (dc, rank) = stream X^T cols, cast, 4 matmuls
        # per kt, drain bf16 into ysb. 32 units spread across stripes.
        xw_units = [(dcc, rk) for dcc in range(DCH) for rk in range(ncores)]
        upers = max(1, (len(xw_units) + S - 1) // S)
        ucnt = [0]

        def emit_xw_unit(u):
            dcc, rk = xw_units[u]
            spc = chunks[dcc]
            wcols = spc * 128
            j0 = (rk * S + cs0[dcc]) * 128
            xt_f = xts.tile([128, FT * wcols], F32)
            for fb in range(FT):
                nc.scalar.dma_start(
                    xt_f[:, fb * wcols:(fb + 1) * wcols],
                    XT_d[fb * 128:(fb + 1) * 128, j0:j0 + wcols],
                )
            xt_b = xtb.tile([128, FT * wcols], BF16)
            Hh = FT * wcols // 2
            if u % 2 == 0:
                nc.scalar.copy(xt_b[:, :Hh], xt_f[:, :Hh])
                nc.vector.tensor_copy(xt_b[:, Hh:], xt_f[:, Hh:])
            else:
                nc.vector.tensor_copy(xt_b[:, :Hh], xt_f[:, :Hh])
                nc.scalar.copy(xt_b[:, Hh:], xt_f[:, Hh:])
            for si in range(spc):
                kt = rk * S + cs0[dcc] + si
                pxw = xwps.tile([128, cdim], F32)
                for fb in range(FT):
                    nc.tensor.matmul(
                        pxw[:],
                        lhsT=xt_b[:, (fb * spc + si) * 128:(fb * spc + si + 1) * 128],
                        rhs=w_bf[:, fb * cdim:(fb + 1) * cdim],
                        start=(fb == 0),
                        stop=(fb == FT - 1),
                    )
                if kt % 2 == 0:
                    nc.vector.tensor_copy(ysb[:, kt * cdim:(kt + 1) * cdim], pxw[:])
                else:
                    nc.scalar.copy(ysb[:, kt * cdim:(kt + 1) * cdim], pxw[:])

        def dgcol(dcc, rk, si):
            return sum(ncores * chunks[i] for i in range(dcc)) + rk * chunks[dcc] + si

        def emit_post(dcc, s_hi):
            """Scale the chunk's XW tiles by gathered d (DVE, due-timed)
            and burst them against streamed stripes."""
            spc = chunks[dcc]
            new_kts = []
            for rk in range(ncores):
                for si in range(spc):
                    kt = rk * S + cs0[dcc] + si
                    new_kts.append(kt)
                    nc.vector.tensor_scalar_mul(
                        ysb[:, kt * cdim:(kt + 1) * cdim],
                        ysb[:, kt * cdim:(kt + 1) * cdim],
                        dg[:, dgcol(dcc, rk, si):dgcol(dcc, rk, si) + 1],
                    )
            for r in range(s_hi + 1):
                emit_burst(r, sorted(new_kts))
            ready_kts.extend(new_kts)

        dc_next = 0
        for s in range(S):
            for c in range(NCH):
                a_ch = ast.tile([128, ch], F32)
                nc.sync.dma_start(
                    a_ch[:], A_d[s * 128:(s + 1) * 128, c * ch:(c + 1) * ch]
                )
                a_bf = abf.tile([128, ch], BF16)
                i = s * NCH + c
                H = ch // 2
                nc.scalar.activation(
                    a_bf[:, :H], a_ch[:, :H], AF.Copy,
                    accum_out=Dacc[:, 2 * i:2 * i + 1],
                )
                nc.vector.tensor_scalar(
                    a_bf[:, H:], a_ch[:, H:], 1.0, 0.0,
                    mybir.AluOpType.mult,
                    mybir.AluOpType.add,
                    accum_out=Dacc[:, 2 * i + 1:2 * i + 2],
                )
                for g in range(NG):
                    pt = tps.tile([128, GRP * 128], BF16)
                    for t in range(GRP):
                        tt = g * GRP + t
                        nc.tensor.transpose(
                            pt[:, t * 128:(t + 1) * 128],
                            a_bf[:, tt * 128:(tt + 1) * 128],
                            ident[:],
                        )
                    kt0 = c * TPC + g * GRP
                    dst = AT[:, (s * KT + kt0) * 128:(s * KT + kt0 + GRP) * 128]
                    if (i * NG + g) % 2 == 0:
                        nc.vector.tensor_copy(dst, pt[:])
                    else:
                        nc.scalar.copy(dst, pt[:])

            while ucnt[0] < min(len(xw_units), (s + 1) * upers):
                emit_xw_unit(ucnt[0])
                ucnt[0] += 1

            if dc_next < DCH and s == cs0[dc_next] + chunks[dc_next] - 1:
                dcc = dc_next
                dc_next += 1
                s0 = cs0[dcc]
                spc = chunks[dcc]
                nc.vector.tensor_reduce(
                    Dsum[:, s0:s0 + spc],
                    Dacc[:, s0 * HACC:(s0 + spc) * HACC].rearrange(
                        "p (s c) -> p s c", s=spc
                    ),
                    axis=mybir.AxisListType.X,
                    op=mybir.AluOpType.add,
                )
                nc.scalar.activation(
                    Dsq[:, s0:s0 + spc], Dsum[:, s0:s0 + spc],
                    AF.Sqrt, bias=bias1[:],
                )
                nc.vector.reciprocal(
                    d_loc[:, s0:s0 + spc], Dsq[:, s0:s0 + spc]
                )
                for sl in range(s0, s0 + spc):
                    nc.vector.tensor_scalar_mul(
                        xw_f32[:, sl * cdim:(sl + 1) * cdim],
                        xw_f32[:, sl * cdim:(sl + 1) * cdim],
                        d_loc[:, sl:sl + 1],
                    )
                # d exchange: bounce + AG + gather-back, all on gpsimd
                # (idle engine => self-timed, no head-of-line hazards).
                nc.gpsimd.dma_start(
                    d_in_d[dcc].rearrange("(s p) -> p s", p=128),
                    d_loc[:, s0:s0 + spc],
                )
                nc.gpsimd.collective_compute(
                    "AllGather",
                    mybir.AluOpType.bypass,
                    replica_groups=groups,
                    ins=[d_in_d[dcc]],
                    outs=[d_out_d[dcc]],
                )
                c0 = dgcol(dcc, 0, 0)
                nc.gpsimd.dma_start(
                    dg[:, c0:c0 + ncores * spc],
                    d_out_d[dcc].rearrange("(k p) -> p k", p=128),
                )
                pending.append((dcc, due_list[dcc]))

            emit_burst(s, ready_kts)

            while pending and pending[0][1] <= s:
                dcc, _ = pending.pop(0)
                emit_post(dcc, s)

        for dcc, _ in pending:
            emit_post(dcc, S - 1)
        pending = []

        # ---- Final: out[r] = relu(d * (bur[r] + Ys_loc[r])) ----
        for r in range(S):
            assert started[r], f"no bursts for stripe {r}"
            dst = bur[:, r * cdim:(r + 1) * cdim]
            nc.vector.tensor_add(dst, dst, xw_f32[:, r * cdim:(r + 1) * cdim])
            ot = outp.tile([128, cdim], F32)
            nc.scalar.activation(ot[:], dst, AF.Relu, scale=d_loc[:, r:r + 1])
            nc.scalar.dma_start(out_d[r * 128:(r + 1) * 128, :], ot[:])

    nc.compile()
    return nc


_NC_CACHE = {}

VARIANT = os.environ.get("GCN_KERNEL_VARIANT", "v6f")


def _get_nc(key=(N, FDIM, CDIM, NCORES)):
    k = (VARIANT, *key)
    if k not in _NC_CACHE:
        builder = {
            "v9": build_v9, "v8": build_v8, "v6f": build_v6f, "v6": build_v6,
            "v4": build_v4, "v3": build_v3,
        }.get(VARIANT, build)
        _NC_CACHE[k] = builder(*key)
    return _NC_CACHE[k]


def kernel(X, A, W, trace=False, **kw):
    X = np.ascontiguousarray(np.asarray(X, dtype=np.float32))
    A = np.ascontiguousarray(np.asarray(A, dtype=np.float32))
    W = np.ascontiguousarray(np.asarray(W, dtype=np.float32))
    n = A.shape[0]
    ncores = NCORES
    R = n // ncores
    if trace:
        _ensure_axon_ntff_hook()
    nc = _get_nc((n, X.shape[1], W.shape[1], ncores))
    XT = None
    if VARIANT == "v9":
        XT = np.ascontiguousarray(X.T)
    in_maps = []
    for i in range(ncores):
        m = {
            "A": A[i * R:(i + 1) * R],
            "X": X[i * R:(i + 1) * R],
            "W": W,
        }
        if XT is not None:
            m["XT"] = XT
        in_maps.append(m)
    res = run_bass_kernel_spmd(nc, in_maps, list(range(ncores)), trace=trace, **kw)
    out = np.concatenate([res.results[i]["out"] for i in range(ncores)], axis=0)
    if trace:
        return out, res
    return out

